# revision 65
# baseline (speedup 1.0000x reference)
"""CTRGC-style GNN message passing kernel for Trainium2 (8 NeuronCores).

Data-parallel over batch N=64: each of 8 cores processes S=8 samples.
Math per sample (fp32 in/out, bf16 internal; rel err ~4e-3 vs the
2e-2 gate):
  xm   = mean_t x                         [C,V]
  x1m/x2m/q/k = (W/T) @ xsum + b          [R,V]   (folded scales)
  d    = tanh(x1m[:,u] - x2m[:,v])        [R,V,V] (stored (v,u) free-major)
  res  = tanh(softmax_v(5*q[:,u]*k[:,v]))
  adj  = W4 @ (d+res) + A + 2*b4          [O,V,V]
  x3   = W3 @ x + b3                      [O,T,V]
  out  = einsum('ouv,otv->otu', adj, x3)  [O,T,U]

Device design (all dtype/bandwidth choices sim-validated against the
TimelineSim cost model; kernel is DMA-bound at ~80% DMA occupancy):
 - x converted to bf16 on host: halves the input DMA (x is only
   consumed through bf16 anyway).  Output DMA'd as bf16 and upcast
   on host.  fp8 was tried and rejected: weight quantization error
   is systematic (~3.6%), no sqrt(C) averaging.
 - 2 samples packed per 128 partitions (block-diag weights).
 - T-sum as a bf16/f16 halving tree over the t-major layout
   (contiguous 2-byte adds -> 2x DVE rate vs strided reduce).
 - x3 = W3@x in bf16 (1 cyc/row), psum copied to bf16 SBUF.
 - restripe bounce through 4 reused per-sample DRAM scratch tensors
   (bf16): scr row (25i+v), cols [0,3328)=x3 (g-major, t), cols
   [3328,4928) = block-diag adjT (off-diag zeroed once); 5 write
   DMAs each for x3/adj per sample (3-dim AP limit), readback in 2
   range-DMAs (x3 region first - it doesn't wait on the adj chain).
 - aggregation per o-group g: ONE bf16 matmul over full t=256,
   lhsT = adjT block [kg,kg] (stationary), rhs = x3 img [kg,256]
   -> psum [(i,u),t], 2 groups per psum bank; copies to osb bf16
   [125,3328] mirroring the img layout; one (split) output DMA per
   sample to DRAM [S,125,3328]; host undoes the (g,i,u,t) permute.
 - software pipeline: per-queue-monotone emission order (aggregation
   of pair p-1 between stage-A1/A2 of pair p; X loads prefetched on
   the Pool/SWDGE queue so no HWDGE DMA ever head-blocks them; x3
   writes before adj writes on SP - readiness order matches queue
   order on every engine).  Engine choices for every copy/DMA were
   swept via CFG knobs with the TimelineSim profiler.
Sim: 117.1us/core vs 307.7us baseline (2.63x).
"""

import numpy as np

S, C, T, V = 8, 64, 256, 25  # per-core samples and dims
O, R = 64, 8
NCORES = 8
NG = 13  # o-groups of 5 (last has 4)
_cache = {}

# emission/scheduling knobs (resolved inside _build_nc)
CFG = {
    "load_eng": "gpsimd",   # engine issuing X loads: sync|scalar|gpsimd
    "b_split": True,        # True: b(p-1,0) before a2, b(p-1,1) after
    "x3w_where": "late",    # x3 restripe writes: early (a2 start) | late
    "x3w_eng": "sync",
    # NOTE: gpsimd cannot access PSUM on hardware; psum->sbuf copy
    # rotations may only use s (Act) and v (DVE).
    "aggcp": "vs",          # agg psum->osb copy engine rotation
    "x3cp": "ssv",           # x3 psum->sbuf copy engine rotation
    "out_eng": "gpsimd",
    "adjw_eng": "sync",
    "rb_eng": "sync",
    "rbd_eng": "scalar",    # diag-block readbacks
    "rb_split": "2way",     # readback range-DMAs: True(3)|"2way"|False
    "out_split": True,      # output DMA in 2 halves
    "load_split": False,    # X load in 2 t-halves, x3 matmuls per half
    "load_split_first": False,   # split only pair 0's load (fill)
    "tail_interleave": False,    # emit b(3,0) between last pair's samples
    "nimg": 3,
    "bounce_dt": "bf16",     # restripe bounce dtype: bf16 | fp8 (e4m3)
    "x_dt": "bf16",          # input x dtype: bf16 | fp8 (e4m3)
    "zi_eng": "scalar",
    "nscr": 3,
    "x3cp0": None,          # override x3 copy rotation for pair 0
}


def _o_of_j(j):
    # adj/x3 partition col order within a sample: j in [0,64)
    if j < 52:
        i, g = j // 13, j % 13
    else:
        i, g = 4, j - 52
    return 5 * g + i


def _build_nc():
    import concourse.bass as bass
    import concourse.bacc as bacc
    import concourse.tile as tile
    import concourse.mybir as mybir
    from concourse.bass import AP  # noqa

    f32 = mybir.dt.float32
    f32r = mybir.dt.float32r
    bf16 = mybir.dt.bfloat16
    bdt = mybir.dt.float8e4 if CFG["bounce_dt"] == "fp8" else bf16
    xdt = mybir.dt.float8e4 if CFG["x_dt"] == "fp8" else bf16
    f16 = mybir.dt.float16
    # Bacc (not raw Bass): its compile() pass legalizes multi-sem waits,
    # which this walrus build rejects ("Too many sync wait commands").
    nc = bacc.Bacc("TRN2", target_bir_lowering=False, debug=False,
                   num_devices=NCORES)

    # x is consumed only through bf16 (x3 matmul + T-sum tree), so the
    # host converts it once and the input DMA moves half the bytes.
    x_d = nc.dram_tensor("x", [S, C, T, V], xdt, kind="ExternalInput").ap()
    w3_d = nc.dram_tensor("w3blk", [128, 128], xdt, kind="ExternalInput").ap()
    wb_d = nc.dram_tensor("wbblk", [128, 64], f32, kind="ExternalInput").ap()
    w4_d = nc.dram_tensor("w4blk", [16, 128], bf16, kind="ExternalInput").ap()
    ar_d = nc.dram_tensor("arep", [128, 625], f32, kind="ExternalInput").ap()
    bb_d = nc.dram_tensor("bbvec", [16, 100], f32, kind="ExternalInput").ap()
    # out is stored permuted: out[s, 25i+u, 256g+t] = y[s, o=5g+i, t, u];
    # the host undoes the permutation and upcasts (cheap numpy ops).
    # bf16 halves the output DMA; the rel-err budget (2e-2) dwarfs the
    # 0.4% bf16 rounding.
    out_d = nc.dram_tensor("out", [S, 125, 3328], bf16,
                           kind="ExternalOutput").ap()
    # Per-sample DRAM scratch for the partition-restripe bounce (SBUF->SBUF
    # restripes with partition-crossing APs are rejected by the BIR verifier;
    # DRAM-side APs are unconstrained). Per-sample tensors keep samples fully
    # independent in dep tracking. Layout per row (25i+v):
    #   cols [0, 3328)    : x3, col = g*256 + t
    #   cols [3328, 4928) : block-diag adjT, col = 3328 + 125g + 25i + u
    IMGW = 4928
    AOFF = 3328
    # scratch tensors reused k%NSCR: the adjT off-diag zeros stay valid
    # across reuse, and sample k's writes only overlap the long-finished
    # readback of sample k-NSCR.
    NSCR = CFG["nscr"]
    scr = [nc.dram_tensor(f"scr{k}", [125, IMGW], bdt, kind="Internal").ap()
           for k in range(NSCR)]

    TV = T * V  # 6400
    cfg = dict(CFG)
    NIMG = cfg["nimg"]

    def _eng(name):
        return {"sync": nc.sync, "scalar": nc.scalar, "vector": nc.vector,
                "gpsimd": nc.gpsimd}[name]

    def _copy(sel, dst, src_):
        if sel == "s":
            nc.scalar.copy(dst, src_)
        elif sel == "v":
            nc.vector.tensor_copy(dst, src_)
        else:
            nc.gpsimd.tensor_copy(dst, src_)

    with tile.TileContext(nc) as tc:
        # The restripe DMAs use partition-crossing inner AP dims; the sim's
        # byte-shadow race detector cannot model those and false-positives.
        tc.race_detector_enabled = False
        from contextlib import ExitStack
        with ExitStack() as ctx:
            consts = ctx.enter_context(tc.tile_pool(name="consts", bufs=1))
            w3sb = consts.tile([128, 128], xdt)
            wbsb = consts.tile([128, 64], f32)
            w4sb = consts.tile([16, 128], bf16)
            arsb = consts.tile([128, 625], f32)
            bbsb = consts.tile([16, 100], f32)
            nc.sync.dma_start(w3sb[:], w3_d)
            nc.sync.dma_start(wbsb[:], wb_d)
            nc.sync.dma_start(w4sb[:], w4_d)
            nc.sync.dma_start(arsb[:], ar_d)
            nc.sync.dma_start(bbsb[:], bb_d)

            # bounce-dtype img tiles (rotating): [125, IMGW]
            img = [consts.tile([125, IMGW], bdt, name=f"img{k}",
                               tag=f"img{k}") for k in range(NIMG)]
            zt = consts.tile([128, 1600], bdt)

            xpool = ctx.enter_context(tc.tile_pool(name="x", bufs=2))
            x3pool = ctx.enter_context(tc.tile_pool(name="x3", bufs=2))
            opool = ctx.enter_context(tc.tile_pool(name="outsb", bufs=2))
            spool = ctx.enter_context(tc.tile_pool(name="small", bufs=2))
            pp = ctx.enter_context(tc.tile_pool(name="ps", bufs=2, space="PSUM"))
            pb = ctx.enter_context(tc.tile_pool(name="psb", bufs=1, space="PSUM"))
            pa = ctx.enter_context(tc.tile_pool(name="psa", bufs=4, space="PSUM"))

            Xs = [None] * 4  # X tiles per pair, for prefetch

            def load_pair(p):
                Xt = xpool.tile([128, TV], xdt, tag="X")
                xflat = x_d[2 * p:2 * p + 2].rearrange("s c t v -> (s c) (t v)")
                if cfg["load_split"] or (p == 0 and cfg["load_split_first"]):
                    _eng(cfg["load_eng"]).dma_start(Xt[:, 0:3200],
                                                    xflat[:, 0:3200])
                    _eng(cfg["load_eng"]).dma_start(Xt[:, 3200:TV],
                                                    xflat[:, 3200:TV])
                else:
                    _eng(cfg["load_eng"]).dma_start(Xt[:], xflat)
                Xs[p] = Xt

            def dram_ap(k, off, dims):
                return bass.AP(scr[k % NSCR].tensor, off,
                               [list(d) for d in dims])

            def stage_a1(p):
                if p + 1 < 4:
                    load_pair(p + 1)
                Xb = Xs[p]
                Xbv = Xb[:].rearrange("p (t v) -> p v t", v=V)  # [128,25,256]

                # T-sum for branch projections (mean folded into weights):
                # halving tree over the t-major layout — contiguous 2-byte
                # adds run at 2-4x DVE rate vs a strided fp32 reduce.
                xt = spool.tile([128, 6400], f16, tag="xtree")
                nc.vector.tensor_add(xt[:, 0:3200], Xb[:, 0:3200],
                                     Xb[:, 3200:6400])
                off = 0
                for wdt in (1600, 800, 400, 200, 100, 50):
                    nc.vector.tensor_add(
                        xt[:, off + 2 * wdt:off + 3 * wdt],
                        xt[:, off:off + wdt], xt[:, off + wdt:off + 2 * wdt])
                    off += 2 * wdt
                xsum = spool.tile([128, V], f32, tag="xsum")
                nc.vector.tensor_add(xsum[:], xt[:, off:off + 25],
                                     xt[:, off + 25:off + 50])

                # x3 = W3blk @ X (v-major free order) in bf16
                x3sb = x3pool.tile([128, TV], bdt, tag="x3sb")
                for j in range(13):
                    w = 2 if j < 12 else 1
                    ps = pp.tile([128, 512], f32, tag="x3ps")
                    if cfg["load_split"]:
                        # per t-half matmuls so compute starts mid-load
                        psv = ps[:, 0:256 * w].rearrange(
                            "p (v h t) -> p v h t", v=w, h=2)
                        for h in range(2):
                            nc.tensor.matmul(
                                psv[:, :, h, :],
                                w3sb[:],
                                Xbv[:, 2 * j:2 * j + w, 128 * h:128 * h + 128],
                                start=True, stop=True)
                    else:
                        nc.tensor.matmul(ps[:, 0:256 * w],
                                         w3sb[:],
                                         Xbv[:, 2 * j:2 * j + w, :],
                                         start=True, stop=True)
                    dst = x3sb[:, 512 * j:512 * j + 256 * w]
                    rot = (cfg["x3cp0"] if (p == 0 and cfg["x3cp0"])
                           else cfg["x3cp"])
                    _copy(rot[j % len(rot)], dst, ps[:, 0:256 * w])

                # branch projections: 4 blocks (x1m,x2m,q,k) all on
                # partitions 0-15 ((s,r)), split along free (25 each).
                # One psum tile per pair holds both bps (cols 640:740)
                # and adj (cols 0:625) to stay within 2 banks.
                bps = pb.tile([128, 768], f32, tag="badj")
                for b in range(4):
                    nc.tensor.matmul(bps[0:16, 640 + 25 * b:640 + 25 * b + 25],
                                     wbsb[:, 16 * b:16 * b + 16], xsum[:],
                                     start=True, stop=True)
                return x3sb, bps

            def x3w(p, x3sb, s):
                k = 2 * p + s
                sb = s * 64
                xe = cfg["x3w_eng"]
                if xe == "alt":
                    xe = ("sync", "scalar")[s]
                for i in range(5):
                    ng = 13 if i < 4 else 12
                    j0 = sb + 13 * i
                    _eng(xe).dma_start(
                        dram_ap(k, 25 * i * IMGW,
                                [[T, ng], [IMGW, V], [1, T]]),
                        x3sb[j0:j0 + ng, :]
                        .rearrange("g (v t) -> g v t", v=V))

            def stage_a2(p, x3sb, bps, after_s0=None):
                if cfg["x3w_where"] == "early":
                    x3w(p, x3sb, 0)
                    x3w(p, x3sb, 1)
                bsb = spool.tile([16, 100], f32, tag="bsb")
                nc.vector.tensor_add(bsb[:], bps[0:16, 640:740], bbsb[:])

                # d = tanh(x1m[u] - x2m[v]);  att = q[u]*k[v]  (free=(v,u))
                x1 = bsb[:, 0:25].unsqueeze(1).broadcast_to([16, V, V])
                x2 = bsb[:, 25:50].unsqueeze(2).broadcast_to([16, V, V])
                qq = bsb[:, 50:75].unsqueeze(1).broadcast_to([16, V, V])
                kk = bsb[:, 75:100].unsqueeze(2).broadcast_to([16, V, V])
                dd = spool.tile([16, 625], f32, tag="dd")
                ddv = dd[:].rearrange("p (v u) -> p v u", v=V)
                nc.vector.tensor_tensor(ddv, x1, x2,
                                        op=mybir.AluOpType.subtract)
                dt_ = spool.tile([16, 625], f32, tag="dt")
                nc.scalar.activation(dt_[:], dd[:],
                                     mybir.ActivationFunctionType.Tanh)
                at = spool.tile([16, 625], f32, tag="at")
                atv = at[:].rearrange("p (v u) -> p v u", v=V)
                nc.vector.tensor_tensor(atv, qq, kk, op=mybir.AluOpType.mult)
                ea = spool.tile([16, 625], f32, tag="ea")
                nc.scalar.activation(ea[:], at[:],
                                     mybir.ActivationFunctionType.Exp)
                den = spool.tile([16, V], f32, tag="den")
                nc.vector.tensor_reduce(
                    den[:], ea[:].rearrange("p (v u) -> p u v", v=V),
                    axis=mybir.AxisListType.X, op=mybir.AluOpType.add)
                rden = spool.tile([16, V], f32, tag="rden")
                nc.vector.reciprocal(rden[:], den[:])
                sm = spool.tile([16, 625], f32, tag="sm")
                nc.vector.tensor_tensor(
                    sm[:].rearrange("p (v u) -> p v u", v=V),
                    ea[:].rearrange("p (v u) -> p v u", v=V),
                    rden[:].unsqueeze(1).broadcast_to([16, V, V]),
                    op=mybir.AluOpType.mult)
                res = spool.tile([16, 625], f32, tag="res")
                nc.scalar.activation(res[:], sm[:],
                                     mybir.ActivationFunctionType.Tanh)
                st = spool.tile([16, 625], bf16, tag="st")
                nc.vector.tensor_add(st[:], dt_[:], res[:])

                # adj = W4blk @ s + (A + 2*b4)   [128,(v,u)], bf16 matmul
                aps_ = bps
                nc.tensor.matmul(aps_[:, 0:512], w4sb[:], st[:, 0:512],
                                 start=True, stop=True)
                nc.tensor.matmul(aps_[:, 512:625], w4sb[:], st[:, 512:625],
                                 start=True, stop=True)
                adjsb = spool.tile([128, 625], bdt, tag="adjsb")
                nc.vector.tensor_add(adjsb[:], aps_[:, 0:625], arsb[:])

                # adj restripe writes + readback into the img tile.
                for s in range(2):
                    k = 2 * p + s
                    sb = s * 64
                    it = img[k % NIMG]
                    if cfg["x3w_where"] != "early":
                        x3w(p, x3sb, s)
                    # DMA APs are limited to 3 dims, so one DMA per i-block.
                    for i in range(5):
                        ng = 13 if i < 4 else 12
                        j0 = sb + 13 * i
                        # adj diag blocks: col 3328 + 125g + 25i + u
                        ae = cfg["adjw_eng"]
                        if ae == "alt":
                            ae = ("sync", "scalar")[s]
                        _eng(ae).dma_start(
                            dram_ap(k, AOFF + 25 * i * IMGW + 25 * i,
                                    [[125, ng], [IMGW, V], [1, V]]),
                            adjsb[j0:j0 + ng, :]
                            .rearrange("g (v u) -> g v u", v=V))
                    if cfg["rb_split"] == "2way":
                        _eng(cfg["rb_eng"]).dma_start(
                            it[:, 0:AOFF],
                            dram_ap(k, 0, [[IMGW, 125], [1, AOFF]]))
                        _eng(cfg["rb_eng"]).dma_start(
                            it[:, AOFF:IMGW],
                            dram_ap(k, AOFF, [[IMGW, 125], [1, IMGW - AOFF]]))
                    elif cfg["rb_split"]:
                        # x3 halves first: they only wait on x3w, so they
                        # stream while adjw waits on the branch chain.
                        _eng(cfg["rb_eng"]).dma_start(
                            it[:, 0:1664],
                            dram_ap(k, 0, [[IMGW, 125], [1, 1664]]))
                        _eng(cfg["rb_eng"]).dma_start(
                            it[:, 1664:AOFF],
                            dram_ap(k, 1664, [[IMGW, 125], [1, AOFF - 1664]]))
                        _eng(cfg["rb_eng"]).dma_start(
                            it[:, AOFF:IMGW],
                            dram_ap(k, AOFF, [[IMGW, 125], [1, IMGW - AOFF]]))
                    else:
                        _eng(cfg["rb_eng"]).dma_start(it[:], scr[k % NSCR])
                    if s == 0 and after_s0 is not None:
                        after_s0()

            def stage_b(p, s):
                # aggregation + output for sample 2p+s.  One matmul per
                # o-group g: lhsT = adjT block (stationary), rhs = x3 img
                # rows -> psum [(i,u), t] with full t=256 moving dim.
                # osb mirrors the img x3 layout [125, (g,t)]; one output
                # DMA per sample, host undoes the (g,i,u,t) permutation.
                k = 2 * p + s
                it = img[k % NIMG]
                osb = opool.tile([125, 3328], bf16, tag="osb")
                for q in range(7):  # psum tiles of 2 groups
                    glist = range(2 * q, min(2 * q + 2, NG))
                    ag = pa.tile([128, 512], f32, tag="aggps")
                    for gi, g in enumerate(glist):
                        kg = 125 if g < 12 else 100
                        nc.tensor.matmul(
                            ag[0:kg, 256 * gi:256 * gi + T],
                            it[0:kg, AOFF + 125 * g:AOFF + 125 * g + kg],
                            it[0:kg, T * g:T * g + T],
                            start=True, stop=True)
                    w = 256 * len(glist)
                    dst = osb[:, 512 * q:512 * q + w]
                    rot = cfg["aggcp"]
                    _copy(rot[q % len(rot)], dst, ag[0:125, 0:w])
                    if cfg["out_split"] and q == 2:
                        _eng(cfg["out_eng"]).dma_start(out_d[k][:, 0:1536],
                                                       osb[:, 0:1536])
                if cfg["out_split"]:
                    _eng(cfg["out_eng"]).dma_start(out_d[k][:, 1536:3328],
                                                   osb[:, 1536:3328])
                else:
                    _eng(cfg["out_eng"]).dma_start(out_d[k], osb[:])

            load_pair(0)
            # one-time zero init of the adjT region of every scratch tensor
            # (off-diagonal blocks stay zero forever); after load 0 so the
            # first X transfer isn't queued behind them.
            nc.vector.memset(zt[:], 0.0)
            for k in range(NSCR):
                _eng(cfg.get("zi_eng", "scalar")).dma_start(
                    bass.AP(scr[k].tensor, AOFF, [[IMGW, 125], [1, 1600]]),
                    zt[0:125, 0:1600])
            for p in range(4):
                x3sb, bps = stage_a1(p)
                if p >= 1:
                    stage_b(p - 1, 0)
                    if not cfg["b_split"]:
                        stage_b(p - 1, 1)
                cb = (lambda: stage_b(3, 0)) \
                    if (p == 3 and cfg["tail_interleave"]) else None
                stage_a2(p, x3sb, bps, after_s0=cb)
                if p >= 1 and cfg["b_split"]:
                    stage_b(p - 1, 1)
            if not cfg["tail_interleave"]:
                stage_b(3, 0)
            stage_b(3, 1)
    nc.compile()
    return nc


def _get_nc():
    if "nc" not in _cache:
        _cache["nc"] = _build_nc()
    return _cache["nc"]


def _host_weights(A, W1, b1, W2, b2, W3, b3, W4, b4, W5, b5, w6, b6, w7, b7):
    f = np.float32
    s5 = np.sqrt(np.float32(5.0))
    Wq = (s5 * w6 * W5).astype(f)
    Wk = (s5 * w7 * W5).astype(f)
    # per-row biases
    bq = (s5 * (w6 * b5 + b6)).astype(f)  # [R]
    bk = (s5 * (w7 * b5 + b7)).astype(f)

    w3blk = np.zeros((128, 128), f)
    perm = np.array([_o_of_j(j) for j in range(64)])  # j -> o
    for s in range(2):
        # columns m = s*64+j hold o=perm[j]:  lhsT[k=c, m] = W3[o, c]
        w3blk[s * 64:(s + 1) * 64, s * 64:(s + 1) * 64] = W3[perm].T

    wbblk = np.zeros((128, 64), f)
    Wset = [W1, W2, Wq, Wk]
    for blk in range(4):
        for s in range(2):
            # columns 16*blk + s*8 + r ; rows k = s*64 + c
            wbblk[s * 64:(s + 1) * 64,
                  16 * blk + s * 8: 16 * blk + s * 8 + 8] = \
                (Wset[blk] / T).T
    bbvec = np.zeros((16, 100), f)
    bset = [b1, b2, bq, bk]
    for blk in range(4):
        for s in range(2):
            bbvec[s * 8:(s + 1) * 8, 25 * blk:25 * blk + 25] = \
                bset[blk][:, None]

    w4blk = np.zeros((16, 128), f)
    for s in range(2):
        # k = s*8 + r ; m = s*64 + j with o=perm[j]
        w4blk[s * 8:(s + 1) * 8, s * 64:(s + 1) * 64] = W4[perm].T

    arep = np.zeros((128, 625), f)
    avu = (A.T).reshape(-1)  # index v*25+u -> A[u,v]
    for s in range(2):
        for j in range(64):
            arep[s * 64 + j, :] = avu + 2.0 * b4[perm[j]]
    return w3blk, wbblk, w4blk, arep, bbvec


def kernel(**inputs):
    import jax  # noqa: F401  (ensures axon/jax devices initialized)
    import ml_dtypes
    from concourse.bass_utils import run_bass_kernel_spmd

    x = np.asarray(inputs["x"], np.float32)
    args = {k: np.asarray(np.float32(inputs[k]))
            for k in ["A", "W1", "b1", "W2", "b2", "W3", "b3", "W4", "b4",
                      "W5", "b5", "w6", "b6", "w7", "b7"]}
    w3blk, wbblk, w4blk, arep, bbvec = _host_weights(**args)
    xnp = (ml_dtypes.float8_e4m3fn if CFG["x_dt"] == "fp8"
           else ml_dtypes.bfloat16)
    w3blk = w3blk.astype(xnp)
    w4blk = w4blk.astype(ml_dtypes.bfloat16)

    nc = _get_nc()
    in_maps = []
    xb = x.astype(xnp)
    for core in range(NCORES):
        in_maps.append({
            "x": np.ascontiguousarray(xb[core * S:(core + 1) * S]),
            "w3blk": w3blk, "wbblk": wbblk, "w4blk": w4blk,
            "arep": arep, "bbvec": bbvec,
        })
    res = run_bass_kernel_spmd(nc, in_maps, list(range(NCORES)))
    _cache["last_results"] = res
    outs = []
    for core in range(NCORES):
        o = np.asarray(res.results[core]["out"]).astype(np.float32)
        # out[s, 25i+u, 256g+t] = y[s, o=5g+i, t, u]; slot o=64 is garbage
        a = o.reshape(S, 5, V, NG, T).transpose(0, 3, 1, 4, 2)
        outs.append(a.reshape(S, 65, T, V)[:, :64])  # -> [S, O, T, U]
    full = np.concatenate(outs, axis=0)

    # b3 correction: out += b3[o] * sum_v adj[o,u,v] — b3 is zero in this
    # problem's setup; assert to be explicit rather than silently wrong.
    assert not np.any(args["b3"]), "kernel assumes b3 == 0"
    return full.astype(np.float32)


# revision 67
# speedup vs baseline: 1.0071x; 1.0071x over previous
"""CTRGC-style GNN message passing kernel for Trainium2 (8 NeuronCores).

Data-parallel over batch N=64: each of 8 cores processes S=8 samples.
Math per sample (fp32 in/out, bf16 internal; rel err ~4e-3 vs the
2e-2 gate):
  xm   = mean_t x                         [C,V]
  x1m/x2m/q/k = (W/T) @ xsum + b          [R,V]   (folded scales)
  d    = tanh(x1m[:,u] - x2m[:,v])        [R,V,V] (stored (v,u) free-major)
  res  = tanh(softmax_v(5*q[:,u]*k[:,v]))
  adj  = W4 @ (d+res) + A + 2*b4          [O,V,V]
  x3   = W3 @ x + b3                      [O,T,V]
  out  = einsum('ouv,otv->otu', adj, x3)  [O,T,U]

Device design (all dtype/bandwidth choices sim-validated against the
TimelineSim cost model; kernel is DMA-bound at ~80% DMA occupancy):
 - x converted to bf16 on host: halves the input DMA (x is only
   consumed through bf16 anyway).  Output DMA'd as bf16 and upcast
   on host.  fp8 was tried and rejected: weight quantization error
   is systematic (~3.6%), no sqrt(C) averaging.
 - 2 samples packed per 128 partitions (block-diag weights).
 - T-sum as a bf16/f16 halving tree over the t-major layout
   (contiguous 2-byte adds -> 2x DVE rate vs strided reduce).
 - x3 = W3@x in bf16 (1 cyc/row), psum copied to bf16 SBUF.
 - restripe bounce through 4 reused per-sample DRAM scratch tensors
   (bf16): scr row (25i+v), cols [0,3328)=x3 (g-major, t), cols
   [3328,4928) = block-diag adjT (off-diag zeroed once); 5 write
   DMAs each for x3/adj per sample (3-dim AP limit), readback in 2
   range-DMAs (x3 region first - it doesn't wait on the adj chain).
 - aggregation per o-group g: ONE bf16 matmul over full t=256,
   lhsT = adjT block [kg,kg] (stationary), rhs = x3 img [kg,256]
   -> psum [(i,u),t], 2 groups per psum bank; copies to osb bf16
   [125,3328] mirroring the img layout; one (split) output DMA per
   sample to DRAM [S,125,3328]; host undoes the (g,i,u,t) permute.
 - software pipeline: per-queue-monotone emission order (aggregation
   of pair p-1 between stage-A1/A2 of pair p; X loads prefetched on
   the Pool/SWDGE queue so no HWDGE DMA ever head-blocks them; x3
   writes before adj writes on SP - readiness order matches queue
   order on every engine).  Engine choices for every copy/DMA were
   swept via CFG knobs with the TimelineSim profiler.
Sim: 116.3us/core vs 307.7us baseline (2.65x).
"""

import numpy as np

S, C, T, V = 8, 64, 256, 25  # per-core samples and dims
O, R = 64, 8
NCORES = 8
NG = 13  # o-groups of 5 (last has 4)
_cache = {}

# emission/scheduling knobs (resolved inside _build_nc)
CFG = {
    "load_eng": "gpsimd",   # engine issuing X loads: sync|scalar|gpsimd
    "b_split": True,        # True: b(p-1,0) before a2, b(p-1,1) after
    "x3w_where": "late",    # x3 restripe writes: early (a2 start) | late
    "x3w_eng": "sync",
    # NOTE: gpsimd cannot access PSUM on hardware; psum->sbuf copy
    # rotations may only use s (Act) and v (DVE).
    "aggcp": "vs",          # agg psum->osb copy engine rotation
    "x3cp": "ssv",           # x3 psum->sbuf copy engine rotation
    "out_eng": "gpsimd",
    "adjw_eng": "sync",
    "rb_eng": "sync",
    "rbd_eng": "scalar",    # diag-block readbacks
    "rb_split": "2way",     # readback range-DMAs: True(3)|"2way"|False
    "out_split": True,      # output DMA in 2 halves
    "load_split": False,    # X load in 2 t-halves, x3 matmuls per half
    "load_split_first": False,   # split only pair 0's load (fill)
    "tail_interleave": False,    # emit b(3,0) between last pair's samples
    "nimg": 3,
    "bounce_dt": "bf16",     # restripe bounce dtype: bf16 | fp8 (e4m3)
    "x_dt": "bf16",          # input x dtype: bf16 | fp8 (e4m3)
    "zi_eng": "scalar",
    "nscr": 3,
    "x3cp0": None,          # override x3 copy rotation for pair 0
    "xpool_bufs": 2,
    "x3pool_bufs": 3,
    "opool_bufs": 4,
    "spool_bufs": 2,
}


def _o_of_j(j):
    # adj/x3 partition col order within a sample: j in [0,64)
    if j < 52:
        i, g = j // 13, j % 13
    else:
        i, g = 4, j - 52
    return 5 * g + i


def _build_nc():
    import concourse.bass as bass
    import concourse.bacc as bacc
    import concourse.tile as tile
    import concourse.mybir as mybir
    from concourse.bass import AP  # noqa

    f32 = mybir.dt.float32
    f32r = mybir.dt.float32r
    bf16 = mybir.dt.bfloat16
    bdt = mybir.dt.float8e4 if CFG["bounce_dt"] == "fp8" else bf16
    xdt = mybir.dt.float8e4 if CFG["x_dt"] == "fp8" else bf16
    f16 = mybir.dt.float16
    # Bacc (not raw Bass): its compile() pass legalizes multi-sem waits,
    # which this walrus build rejects ("Too many sync wait commands").
    nc = bacc.Bacc("TRN2", target_bir_lowering=False, debug=False,
                   num_devices=NCORES)

    # x is consumed only through bf16 (x3 matmul + T-sum tree), so the
    # host converts it once and the input DMA moves half the bytes.
    x_d = nc.dram_tensor("x", [S, C, T, V], xdt, kind="ExternalInput").ap()
    w3_d = nc.dram_tensor("w3blk", [128, 128], xdt, kind="ExternalInput").ap()
    wb_d = nc.dram_tensor("wbblk", [128, 64], f32, kind="ExternalInput").ap()
    w4_d = nc.dram_tensor("w4blk", [16, 128], bf16, kind="ExternalInput").ap()
    ar_d = nc.dram_tensor("arep", [128, 625], f32, kind="ExternalInput").ap()
    bb_d = nc.dram_tensor("bbvec", [16, 100], f32, kind="ExternalInput").ap()
    # out is stored permuted: out[s, 25i+u, 256g+t] = y[s, o=5g+i, t, u];
    # the host undoes the permutation and upcasts (cheap numpy ops).
    # bf16 halves the output DMA; the rel-err budget (2e-2) dwarfs the
    # 0.4% bf16 rounding.
    out_d = nc.dram_tensor("out", [S, 125, 3328], bf16,
                           kind="ExternalOutput").ap()
    # Per-sample DRAM scratch for the partition-restripe bounce (SBUF->SBUF
    # restripes with partition-crossing APs are rejected by the BIR verifier;
    # DRAM-side APs are unconstrained). Per-sample tensors keep samples fully
    # independent in dep tracking. Layout per row (25i+v):
    #   cols [0, 3328)    : x3, col = g*256 + t
    #   cols [3328, 4928) : block-diag adjT, col = 3328 + 125g + 25i + u
    IMGW = 4928
    AOFF = 3328
    # scratch tensors reused k%NSCR: the adjT off-diag zeros stay valid
    # across reuse, and sample k's writes only overlap the long-finished
    # readback of sample k-NSCR.
    NSCR = CFG["nscr"]
    scr = [nc.dram_tensor(f"scr{k}", [125, IMGW], bdt, kind="Internal").ap()
           for k in range(NSCR)]

    TV = T * V  # 6400
    cfg = dict(CFG)
    NIMG = cfg["nimg"]

    def _eng(name):
        return {"sync": nc.sync, "scalar": nc.scalar, "vector": nc.vector,
                "gpsimd": nc.gpsimd}[name]

    def _copy(sel, dst, src_):
        if sel == "s":
            nc.scalar.copy(dst, src_)
        elif sel == "v":
            nc.vector.tensor_copy(dst, src_)
        else:
            nc.gpsimd.tensor_copy(dst, src_)

    with tile.TileContext(nc) as tc:
        # The restripe DMAs use partition-crossing inner AP dims; the sim's
        # byte-shadow race detector cannot model those and false-positives.
        tc.race_detector_enabled = False
        from contextlib import ExitStack
        with ExitStack() as ctx:
            consts = ctx.enter_context(tc.tile_pool(name="consts", bufs=1))
            w3sb = consts.tile([128, 128], xdt)
            wbsb = consts.tile([128, 64], f32)
            w4sb = consts.tile([16, 128], bf16)
            arsb = consts.tile([128, 625], f32)
            bbsb = consts.tile([16, 100], f32)
            nc.sync.dma_start(w3sb[:], w3_d)
            nc.sync.dma_start(wbsb[:], wb_d)
            nc.sync.dma_start(w4sb[:], w4_d)
            nc.sync.dma_start(arsb[:], ar_d)
            nc.sync.dma_start(bbsb[:], bb_d)

            # bounce-dtype img tiles (rotating): [125, IMGW]
            img = [consts.tile([125, IMGW], bdt, name=f"img{k}",
                               tag=f"img{k}") for k in range(NIMG)]
            zt = consts.tile([128, 1600], bdt)

            xpool = ctx.enter_context(
                tc.tile_pool(name="x", bufs=cfg["xpool_bufs"]))
            x3pool = ctx.enter_context(
                tc.tile_pool(name="x3", bufs=cfg["x3pool_bufs"]))
            opool = ctx.enter_context(
                tc.tile_pool(name="outsb", bufs=cfg["opool_bufs"]))
            spool = ctx.enter_context(
                tc.tile_pool(name="small", bufs=cfg["spool_bufs"]))
            pp = ctx.enter_context(tc.tile_pool(name="ps", bufs=2, space="PSUM"))
            pb = ctx.enter_context(tc.tile_pool(name="psb", bufs=1, space="PSUM"))
            pa = ctx.enter_context(tc.tile_pool(name="psa", bufs=4, space="PSUM"))

            Xs = [None] * 4  # X tiles per pair, for prefetch

            def load_pair(p):
                Xt = xpool.tile([128, TV], xdt, tag="X")
                xflat = x_d[2 * p:2 * p + 2].rearrange("s c t v -> (s c) (t v)")
                if cfg["load_split"] or (p == 0 and cfg["load_split_first"]):
                    _eng(cfg["load_eng"]).dma_start(Xt[:, 0:3200],
                                                    xflat[:, 0:3200])
                    _eng(cfg["load_eng"]).dma_start(Xt[:, 3200:TV],
                                                    xflat[:, 3200:TV])
                else:
                    _eng(cfg["load_eng"]).dma_start(Xt[:], xflat)
                Xs[p] = Xt

            def dram_ap(k, off, dims):
                return bass.AP(scr[k % NSCR].tensor, off,
                               [list(d) for d in dims])

            def stage_a1(p):
                if p + 1 < 4:
                    load_pair(p + 1)
                Xb = Xs[p]
                Xbv = Xb[:].rearrange("p (t v) -> p v t", v=V)  # [128,25,256]

                # T-sum for branch projections (mean folded into weights):
                # halving tree over the t-major layout — contiguous 2-byte
                # adds run at 2-4x DVE rate vs a strided fp32 reduce.
                xt = spool.tile([128, 6400], f16, tag="xtree")
                nc.vector.tensor_add(xt[:, 0:3200], Xb[:, 0:3200],
                                     Xb[:, 3200:6400])
                off = 0
                for wdt in (1600, 800, 400, 200, 100, 50):
                    nc.vector.tensor_add(
                        xt[:, off + 2 * wdt:off + 3 * wdt],
                        xt[:, off:off + wdt], xt[:, off + wdt:off + 2 * wdt])
                    off += 2 * wdt
                xsum = spool.tile([128, V], f32, tag="xsum")
                nc.vector.tensor_add(xsum[:], xt[:, off:off + 25],
                                     xt[:, off + 25:off + 50])

                # x3 = W3blk @ X (v-major free order) in bf16
                x3sb = x3pool.tile([128, TV], bdt, tag="x3sb")
                for j in range(13):
                    w = 2 if j < 12 else 1
                    ps = pp.tile([128, 512], f32, tag="x3ps")
                    if cfg["load_split"]:
                        # per t-half matmuls so compute starts mid-load
                        psv = ps[:, 0:256 * w].rearrange(
                            "p (v h t) -> p v h t", v=w, h=2)
                        for h in range(2):
                            nc.tensor.matmul(
                                psv[:, :, h, :],
                                w3sb[:],
                                Xbv[:, 2 * j:2 * j + w, 128 * h:128 * h + 128],
                                start=True, stop=True)
                    else:
                        nc.tensor.matmul(ps[:, 0:256 * w],
                                         w3sb[:],
                                         Xbv[:, 2 * j:2 * j + w, :],
                                         start=True, stop=True)
                    dst = x3sb[:, 512 * j:512 * j + 256 * w]
                    rot = (cfg["x3cp0"] if (p == 0 and cfg["x3cp0"])
                           else cfg["x3cp"])
                    _copy(rot[j % len(rot)], dst, ps[:, 0:256 * w])

                # branch projections: 4 blocks (x1m,x2m,q,k) all on
                # partitions 0-15 ((s,r)), split along free (25 each).
                # One psum tile per pair holds both bps (cols 640:740)
                # and adj (cols 0:625) to stay within 2 banks.
                bps = pb.tile([128, 768], f32, tag="badj")
                for b in range(4):
                    nc.tensor.matmul(bps[0:16, 640 + 25 * b:640 + 25 * b + 25],
                                     wbsb[:, 16 * b:16 * b + 16], xsum[:],
                                     start=True, stop=True)
                return x3sb, bps

            def x3w(p, x3sb, s):
                k = 2 * p + s
                sb = s * 64
                xe = cfg["x3w_eng"]
                if xe == "alt":
                    xe = ("sync", "scalar")[s]
                for i in range(5):
                    ng = 13 if i < 4 else 12
                    j0 = sb + 13 * i
                    _eng(xe).dma_start(
                        dram_ap(k, 25 * i * IMGW,
                                [[T, ng], [IMGW, V], [1, T]]),
                        x3sb[j0:j0 + ng, :]
                        .rearrange("g (v t) -> g v t", v=V))

            def stage_a2(p, x3sb, bps, after_s0=None):
                if cfg["x3w_where"] == "early":
                    x3w(p, x3sb, 0)
                    x3w(p, x3sb, 1)
                bsb = spool.tile([16, 100], f32, tag="bsb")
                nc.vector.tensor_add(bsb[:], bps[0:16, 640:740], bbsb[:])

                # d = tanh(x1m[u] - x2m[v]);  att = q[u]*k[v]  (free=(v,u))
                x1 = bsb[:, 0:25].unsqueeze(1).broadcast_to([16, V, V])
                x2 = bsb[:, 25:50].unsqueeze(2).broadcast_to([16, V, V])
                qq = bsb[:, 50:75].unsqueeze(1).broadcast_to([16, V, V])
                kk = bsb[:, 75:100].unsqueeze(2).broadcast_to([16, V, V])
                dd = spool.tile([16, 625], f32, tag="dd")
                ddv = dd[:].rearrange("p (v u) -> p v u", v=V)
                nc.vector.tensor_tensor(ddv, x1, x2,
                                        op=mybir.AluOpType.subtract)
                dt_ = spool.tile([16, 625], f32, tag="dt")
                nc.scalar.activation(dt_[:], dd[:],
                                     mybir.ActivationFunctionType.Tanh)
                at = spool.tile([16, 625], f32, tag="at")
                atv = at[:].rearrange("p (v u) -> p v u", v=V)
                nc.vector.tensor_tensor(atv, qq, kk, op=mybir.AluOpType.mult)
                ea = spool.tile([16, 625], f32, tag="ea")
                nc.scalar.activation(ea[:], at[:],
                                     mybir.ActivationFunctionType.Exp)
                den = spool.tile([16, V], f32, tag="den")
                nc.vector.tensor_reduce(
                    den[:], ea[:].rearrange("p (v u) -> p u v", v=V),
                    axis=mybir.AxisListType.X, op=mybir.AluOpType.add)
                rden = spool.tile([16, V], f32, tag="rden")
                nc.vector.reciprocal(rden[:], den[:])
                sm = spool.tile([16, 625], f32, tag="sm")
                nc.vector.tensor_tensor(
                    sm[:].rearrange("p (v u) -> p v u", v=V),
                    ea[:].rearrange("p (v u) -> p v u", v=V),
                    rden[:].unsqueeze(1).broadcast_to([16, V, V]),
                    op=mybir.AluOpType.mult)
                res = spool.tile([16, 625], f32, tag="res")
                nc.scalar.activation(res[:], sm[:],
                                     mybir.ActivationFunctionType.Tanh)
                st = spool.tile([16, 625], bf16, tag="st")
                nc.vector.tensor_add(st[:], dt_[:], res[:])

                # adj = W4blk @ s + (A + 2*b4)   [128,(v,u)], bf16 matmul
                aps_ = bps
                nc.tensor.matmul(aps_[:, 0:512], w4sb[:], st[:, 0:512],
                                 start=True, stop=True)
                nc.tensor.matmul(aps_[:, 512:625], w4sb[:], st[:, 512:625],
                                 start=True, stop=True)
                adjsb = spool.tile([128, 625], bdt, tag="adjsb")
                nc.vector.tensor_add(adjsb[:], aps_[:, 0:625], arsb[:])

                # adj restripe writes + readback into the img tile.
                for s in range(2):
                    k = 2 * p + s
                    sb = s * 64
                    it = img[k % NIMG]
                    if cfg["x3w_where"] != "early":
                        x3w(p, x3sb, s)
                    # DMA APs are limited to 3 dims, so one DMA per i-block.
                    for i in range(5):
                        ng = 13 if i < 4 else 12
                        j0 = sb + 13 * i
                        # adj diag blocks: col 3328 + 125g + 25i + u
                        ae = cfg["adjw_eng"]
                        if ae == "alt":
                            ae = ("sync", "scalar")[s]
                        _eng(ae).dma_start(
                            dram_ap(k, AOFF + 25 * i * IMGW + 25 * i,
                                    [[125, ng], [IMGW, V], [1, V]]),
                            adjsb[j0:j0 + ng, :]
                            .rearrange("g (v u) -> g v u", v=V))
                    if cfg["rb_split"] == "2way":
                        _eng(cfg["rb_eng"]).dma_start(
                            it[:, 0:AOFF],
                            dram_ap(k, 0, [[IMGW, 125], [1, AOFF]]))
                        _eng(cfg["rb_eng"]).dma_start(
                            it[:, AOFF:IMGW],
                            dram_ap(k, AOFF, [[IMGW, 125], [1, IMGW - AOFF]]))
                    elif cfg["rb_split"]:
                        # x3 halves first: they only wait on x3w, so they
                        # stream while adjw waits on the branch chain.
                        _eng(cfg["rb_eng"]).dma_start(
                            it[:, 0:1664],
                            dram_ap(k, 0, [[IMGW, 125], [1, 1664]]))
                        _eng(cfg["rb_eng"]).dma_start(
                            it[:, 1664:AOFF],
                            dram_ap(k, 1664, [[IMGW, 125], [1, AOFF - 1664]]))
                        _eng(cfg["rb_eng"]).dma_start(
                            it[:, AOFF:IMGW],
                            dram_ap(k, AOFF, [[IMGW, 125], [1, IMGW - AOFF]]))
                    else:
                        _eng(cfg["rb_eng"]).dma_start(it[:], scr[k % NSCR])
                    if s == 0 and after_s0 is not None:
                        after_s0()

            def stage_b(p, s):
                # aggregation + output for sample 2p+s.  One matmul per
                # o-group g: lhsT = adjT block (stationary), rhs = x3 img
                # rows -> psum [(i,u), t] with full t=256 moving dim.
                # osb mirrors the img x3 layout [125, (g,t)]; one output
                # DMA per sample, host undoes the (g,i,u,t) permutation.
                k = 2 * p + s
                it = img[k % NIMG]
                osb = opool.tile([125, 3328], bf16, tag="osb")
                for q in range(7):  # psum tiles of 2 groups
                    glist = range(2 * q, min(2 * q + 2, NG))
                    ag = pa.tile([128, 512], f32, tag="aggps")
                    for gi, g in enumerate(glist):
                        kg = 125 if g < 12 else 100
                        nc.tensor.matmul(
                            ag[0:kg, 256 * gi:256 * gi + T],
                            it[0:kg, AOFF + 125 * g:AOFF + 125 * g + kg],
                            it[0:kg, T * g:T * g + T],
                            start=True, stop=True)
                    w = 256 * len(glist)
                    dst = osb[:, 512 * q:512 * q + w]
                    rot = cfg["aggcp"]
                    _copy(rot[q % len(rot)], dst, ag[0:125, 0:w])
                    if cfg["out_split"] and q == 2:
                        _eng(cfg["out_eng"]).dma_start(out_d[k][:, 0:1536],
                                                       osb[:, 0:1536])
                if cfg["out_split"]:
                    _eng(cfg["out_eng"]).dma_start(out_d[k][:, 1536:3328],
                                                   osb[:, 1536:3328])
                else:
                    _eng(cfg["out_eng"]).dma_start(out_d[k], osb[:])

            load_pair(0)
            # one-time zero init of the adjT region of every scratch tensor
            # (off-diagonal blocks stay zero forever); after load 0 so the
            # first X transfer isn't queued behind them.
            nc.vector.memset(zt[:], 0.0)
            for k in range(NSCR):
                _eng(cfg.get("zi_eng", "scalar")).dma_start(
                    bass.AP(scr[k].tensor, AOFF, [[IMGW, 125], [1, 1600]]),
                    zt[0:125, 0:1600])
            for p in range(4):
                x3sb, bps = stage_a1(p)
                if p >= 1:
                    stage_b(p - 1, 0)
                    if not cfg["b_split"]:
                        stage_b(p - 1, 1)
                cb = (lambda: stage_b(3, 0)) \
                    if (p == 3 and cfg["tail_interleave"]) else None
                stage_a2(p, x3sb, bps, after_s0=cb)
                if p >= 1 and cfg["b_split"]:
                    stage_b(p - 1, 1)
            if not cfg["tail_interleave"]:
                stage_b(3, 0)
            stage_b(3, 1)
    nc.compile()
    return nc


def _get_nc():
    if "nc" not in _cache:
        _cache["nc"] = _build_nc()
    return _cache["nc"]


def _host_weights(A, W1, b1, W2, b2, W3, b3, W4, b4, W5, b5, w6, b6, w7, b7):
    f = np.float32
    s5 = np.sqrt(np.float32(5.0))
    Wq = (s5 * w6 * W5).astype(f)
    Wk = (s5 * w7 * W5).astype(f)
    # per-row biases
    bq = (s5 * (w6 * b5 + b6)).astype(f)  # [R]
    bk = (s5 * (w7 * b5 + b7)).astype(f)

    w3blk = np.zeros((128, 128), f)
    perm = np.array([_o_of_j(j) for j in range(64)])  # j -> o
    for s in range(2):
        # columns m = s*64+j hold o=perm[j]:  lhsT[k=c, m] = W3[o, c]
        w3blk[s * 64:(s + 1) * 64, s * 64:(s + 1) * 64] = W3[perm].T

    wbblk = np.zeros((128, 64), f)
    Wset = [W1, W2, Wq, Wk]
    for blk in range(4):
        for s in range(2):
            # columns 16*blk + s*8 + r ; rows k = s*64 + c
            wbblk[s * 64:(s + 1) * 64,
                  16 * blk + s * 8: 16 * blk + s * 8 + 8] = \
                (Wset[blk] / T).T
    bbvec = np.zeros((16, 100), f)
    bset = [b1, b2, bq, bk]
    for blk in range(4):
        for s in range(2):
            bbvec[s * 8:(s + 1) * 8, 25 * blk:25 * blk + 25] = \
                bset[blk][:, None]

    w4blk = np.zeros((16, 128), f)
    for s in range(2):
        # k = s*8 + r ; m = s*64 + j with o=perm[j]
        w4blk[s * 8:(s + 1) * 8, s * 64:(s + 1) * 64] = W4[perm].T

    arep = np.zeros((128, 625), f)
    avu = (A.T).reshape(-1)  # index v*25+u -> A[u,v]
    for s in range(2):
        for j in range(64):
            arep[s * 64 + j, :] = avu + 2.0 * b4[perm[j]]
    return w3blk, wbblk, w4blk, arep, bbvec


def kernel(**inputs):
    import jax  # noqa: F401  (ensures axon/jax devices initialized)
    import ml_dtypes
    from concourse.bass_utils import run_bass_kernel_spmd

    x = np.asarray(inputs["x"], np.float32)
    args = {k: np.asarray(np.float32(inputs[k]))
            for k in ["A", "W1", "b1", "W2", "b2", "W3", "b3", "W4", "b4",
                      "W5", "b5", "w6", "b6", "w7", "b7"]}
    w3blk, wbblk, w4blk, arep, bbvec = _host_weights(**args)
    xnp = (ml_dtypes.float8_e4m3fn if CFG["x_dt"] == "fp8"
           else ml_dtypes.bfloat16)
    w3blk = w3blk.astype(xnp)
    w4blk = w4blk.astype(ml_dtypes.bfloat16)

    nc = _get_nc()
    in_maps = []
    xb = x.astype(xnp)
    for core in range(NCORES):
        in_maps.append({
            "x": np.ascontiguousarray(xb[core * S:(core + 1) * S]),
            "w3blk": w3blk, "wbblk": wbblk, "w4blk": w4blk,
            "arep": arep, "bbvec": bbvec,
        })
    res = run_bass_kernel_spmd(nc, in_maps, list(range(NCORES)))
    _cache["last_results"] = res
    outs = []
    for core in range(NCORES):
        o = np.asarray(res.results[core]["out"]).astype(np.float32)
        # out[s, 25i+u, 256g+t] = y[s, o=5g+i, t, u]; slot o=64 is garbage
        a = o.reshape(S, 5, V, NG, T).transpose(0, 3, 1, 4, 2)
        outs.append(a.reshape(S, 65, T, V)[:, :64])  # -> [S, O, T, U]
    full = np.concatenate(outs, axis=0)

    # b3 correction: out += b3[o] * sum_v adj[o,u,v] — b3 is zero in this
    # problem's setup; assert to be explicit rather than silently wrong.
    assert not np.any(args["b3"]), "kernel assumes b3 == 0"
    return full.astype(np.float32)


# revision 69
# speedup vs baseline: 1.0153x; 1.0082x over previous
"""CTRGC-style GNN message passing kernel for Trainium2 (8 NeuronCores).

Data-parallel over batch N=64: each of 8 cores processes S=8 samples.
Math per sample (fp32 in/out, bf16 internal; rel err ~4e-3 vs the
2e-2 gate):
  xm   = mean_t x                         [C,V]
  x1m/x2m/q/k = (W/T) @ xsum + b          [R,V]   (folded scales)
  d    = tanh(x1m[:,u] - x2m[:,v])        [R,V,V] (stored (v,u) free-major)
  res  = tanh(softmax_v(5*q[:,u]*k[:,v]))
  adj  = W4 @ (d+res) + A + 2*b4          [O,V,V]
  x3   = W3 @ x + b3                      [O,T,V]
  out  = einsum('ouv,otv->otu', adj, x3)  [O,T,U]

Device design (all dtype/bandwidth choices sim-validated against the
TimelineSim cost model; kernel is DMA-bound at ~80% DMA occupancy):
 - x converted to bf16 on host: halves the input DMA (x is only
   consumed through bf16 anyway).  Output DMA'd as bf16 and upcast
   on host.  fp8 was tried and rejected: weight quantization error
   is systematic (~3.6%), no sqrt(C) averaging.
 - 2 samples packed per 128 partitions (block-diag weights).
 - T-sum as a bf16/f16 halving tree over the t-major layout
   (contiguous 2-byte adds -> 2x DVE rate vs strided reduce).
 - x3 = W3@x in bf16 (1 cyc/row), psum copied to bf16 SBUF.
 - restripe bounce through 4 reused per-sample DRAM scratch tensors
   (bf16): scr row (25i+v), cols [0,3328)=x3 (g-major, t), cols
   [3328,4928) = block-diag adjT (off-diag zeroed once); 5 write
   DMAs each for x3/adj per sample (3-dim AP limit), readback in 2
   range-DMAs (x3 region first - it doesn't wait on the adj chain).
 - aggregation per o-group g: ONE bf16 matmul over full t=256,
   lhsT = adjT block [kg,kg] (stationary), rhs = x3 img [kg,256]
   -> psum [(i,u),t], 2 groups per psum bank; copies to osb bf16
   [125,3328] mirroring the img layout; one (split) output DMA per
   sample to DRAM [S,125,3328]; host undoes the (g,i,u,t) permute.
 - software pipeline: per-queue-monotone emission order (aggregation
   of pair p-1 between stage-A1/A2 of pair p; X loads prefetched on
   the Pool/SWDGE queue so no HWDGE DMA ever head-blocks them; x3
   writes before adj writes on SP - readiness order matches queue
   order on every engine).  Engine choices for every copy/DMA were
   swept via CFG knobs with the TimelineSim profiler.
Sim: 115.4us/core vs 307.7us baseline (2.67x).
"""

import numpy as np

S, C, T, V = 8, 64, 256, 25  # per-core samples and dims
O, R = 64, 8
NCORES = 8
NG = 13  # o-groups of 5 (last has 4)
_cache = {}

# emission/scheduling knobs (resolved inside _build_nc)
CFG = {
    "load_eng": "gpsimd",   # engine issuing X loads: sync|scalar|gpsimd
    "b_split": True,        # True: b(p-1,0) before a2, b(p-1,1) after
    "x3w_where": "late",    # x3 restripe writes: early (a2 start) | late
    "x3w_eng": "sync",
    # NOTE: gpsimd cannot access PSUM on hardware; psum->sbuf copy
    # rotations may only use s (Act) and v (DVE).
    "aggcp": "vs",          # agg psum->osb copy engine rotation
    "x3cp": "sv",           # x3 psum->sbuf copy engine rotation
    "out_eng": "gpsimd",
    "adjw_eng": "sync",
    "rb_eng": "sync",
    "rbd_eng": "scalar",    # diag-block readbacks
    "rb_split": "2way",     # readback range-DMAs: True(3)|"2way"|False
    "out_split": True,      # output DMA in 2 halves
    "load_split": False,    # X load in 2 t-halves, x3 matmuls per half
    "load_split_first": False,   # split only pair 0's load (fill)
    "tail_interleave": False,    # emit b(3,0) between last pair's samples
    "nimg": 3,
    "bounce_dt": "bf16",     # restripe bounce dtype: bf16 | fp8 (e4m3)
    "x_dt": "bf16",          # input x dtype: bf16 | fp8 (e4m3)
    "zi_eng": "scalar",
    "nscr": 3,
    "x3cp0": None,          # override x3 copy rotation for pair 0
    "xpool_bufs": 2,
    "x3pool_bufs": 3,
    "opool_bufs": 4,
    "spool_bufs": 2,
    "pp_bufs": 4,
    "pa_bufs": 2,
}


def _o_of_j(j):
    # adj/x3 partition col order within a sample: j in [0,64)
    if j < 52:
        i, g = j // 13, j % 13
    else:
        i, g = 4, j - 52
    return 5 * g + i


def _build_nc():
    import concourse.bass as bass
    import concourse.bacc as bacc
    import concourse.tile as tile
    import concourse.mybir as mybir
    from concourse.bass import AP  # noqa

    f32 = mybir.dt.float32
    f32r = mybir.dt.float32r
    bf16 = mybir.dt.bfloat16
    bdt = mybir.dt.float8e4 if CFG["bounce_dt"] == "fp8" else bf16
    xdt = mybir.dt.float8e4 if CFG["x_dt"] == "fp8" else bf16
    f16 = mybir.dt.float16
    # Bacc (not raw Bass): its compile() pass legalizes multi-sem waits,
    # which this walrus build rejects ("Too many sync wait commands").
    nc = bacc.Bacc("TRN2", target_bir_lowering=False, debug=False,
                   num_devices=NCORES)

    # x is consumed only through bf16 (x3 matmul + T-sum tree), so the
    # host converts it once and the input DMA moves half the bytes.
    x_d = nc.dram_tensor("x", [S, C, T, V], xdt, kind="ExternalInput").ap()
    w3_d = nc.dram_tensor("w3blk", [128, 128], xdt, kind="ExternalInput").ap()
    wb_d = nc.dram_tensor("wbblk", [128, 64], f32, kind="ExternalInput").ap()
    w4_d = nc.dram_tensor("w4blk", [16, 128], bf16, kind="ExternalInput").ap()
    ar_d = nc.dram_tensor("arep", [128, 625], f32, kind="ExternalInput").ap()
    bb_d = nc.dram_tensor("bbvec", [16, 100], f32, kind="ExternalInput").ap()
    # out is stored permuted: out[s, 25i+u, 256g+t] = y[s, o=5g+i, t, u];
    # the host undoes the permutation and upcasts (cheap numpy ops).
    # bf16 halves the output DMA; the rel-err budget (2e-2) dwarfs the
    # 0.4% bf16 rounding.
    out_d = nc.dram_tensor("out", [S, 125, 3328], bf16,
                           kind="ExternalOutput").ap()
    # Per-sample DRAM scratch for the partition-restripe bounce (SBUF->SBUF
    # restripes with partition-crossing APs are rejected by the BIR verifier;
    # DRAM-side APs are unconstrained). Per-sample tensors keep samples fully
    # independent in dep tracking. Layout per row (25i+v):
    #   cols [0, 3328)    : x3, col = g*256 + t
    #   cols [3328, 4928) : block-diag adjT, col = 3328 + 125g + 25i + u
    IMGW = 4928
    AOFF = 3328
    # scratch tensors reused k%NSCR: the adjT off-diag zeros stay valid
    # across reuse, and sample k's writes only overlap the long-finished
    # readback of sample k-NSCR.
    NSCR = CFG["nscr"]
    scr = [nc.dram_tensor(f"scr{k}", [125, IMGW], bdt, kind="Internal").ap()
           for k in range(NSCR)]

    TV = T * V  # 6400
    cfg = dict(CFG)
    NIMG = cfg["nimg"]

    def _eng(name):
        return {"sync": nc.sync, "scalar": nc.scalar, "vector": nc.vector,
                "gpsimd": nc.gpsimd}[name]

    def _copy(sel, dst, src_):
        if sel == "s":
            nc.scalar.copy(dst, src_)
        elif sel == "v":
            nc.vector.tensor_copy(dst, src_)
        else:
            nc.gpsimd.tensor_copy(dst, src_)

    with tile.TileContext(nc) as tc:
        # The restripe DMAs use partition-crossing inner AP dims; the sim's
        # byte-shadow race detector cannot model those and false-positives.
        tc.race_detector_enabled = False
        from contextlib import ExitStack
        with ExitStack() as ctx:
            consts = ctx.enter_context(tc.tile_pool(name="consts", bufs=1))
            w3sb = consts.tile([128, 128], xdt)
            wbsb = consts.tile([128, 64], f32)
            w4sb = consts.tile([16, 128], bf16)
            arsb = consts.tile([128, 625], f32)
            bbsb = consts.tile([16, 100], f32)
            nc.sync.dma_start(w3sb[:], w3_d)
            nc.sync.dma_start(wbsb[:], wb_d)
            nc.sync.dma_start(w4sb[:], w4_d)
            nc.sync.dma_start(arsb[:], ar_d)
            nc.sync.dma_start(bbsb[:], bb_d)

            # bounce-dtype img tiles (rotating): [125, IMGW]
            img = [consts.tile([125, IMGW], bdt, name=f"img{k}",
                               tag=f"img{k}") for k in range(NIMG)]
            zt = consts.tile([128, 1600], bdt)

            xpool = ctx.enter_context(
                tc.tile_pool(name="x", bufs=cfg["xpool_bufs"]))
            x3pool = ctx.enter_context(
                tc.tile_pool(name="x3", bufs=cfg["x3pool_bufs"]))
            opool = ctx.enter_context(
                tc.tile_pool(name="outsb", bufs=cfg["opool_bufs"]))
            spool = ctx.enter_context(
                tc.tile_pool(name="small", bufs=cfg["spool_bufs"]))
            pp = ctx.enter_context(
                tc.tile_pool(name="ps", bufs=cfg["pp_bufs"], space="PSUM"))
            pb = ctx.enter_context(
                tc.tile_pool(name="psb", bufs=1, space="PSUM"))
            pa = ctx.enter_context(
                tc.tile_pool(name="psa", bufs=cfg["pa_bufs"], space="PSUM"))

            Xs = [None] * 4  # X tiles per pair, for prefetch

            def load_pair(p):
                Xt = xpool.tile([128, TV], xdt, tag="X")
                xflat = x_d[2 * p:2 * p + 2].rearrange("s c t v -> (s c) (t v)")
                if cfg["load_split"] or (p == 0 and cfg["load_split_first"]):
                    _eng(cfg["load_eng"]).dma_start(Xt[:, 0:3200],
                                                    xflat[:, 0:3200])
                    _eng(cfg["load_eng"]).dma_start(Xt[:, 3200:TV],
                                                    xflat[:, 3200:TV])
                else:
                    _eng(cfg["load_eng"]).dma_start(Xt[:], xflat)
                Xs[p] = Xt

            def dram_ap(k, off, dims):
                return bass.AP(scr[k % NSCR].tensor, off,
                               [list(d) for d in dims])

            def stage_a1(p):
                if p + 1 < 4:
                    load_pair(p + 1)
                Xb = Xs[p]
                Xbv = Xb[:].rearrange("p (t v) -> p v t", v=V)  # [128,25,256]

                # T-sum for branch projections (mean folded into weights):
                # halving tree over the t-major layout — contiguous 2-byte
                # adds run at 2-4x DVE rate vs a strided fp32 reduce.
                xt = spool.tile([128, 6400], f16, tag="xtree")
                nc.vector.tensor_add(xt[:, 0:3200], Xb[:, 0:3200],
                                     Xb[:, 3200:6400])
                off = 0
                for wdt in (1600, 800, 400, 200, 100, 50):
                    nc.vector.tensor_add(
                        xt[:, off + 2 * wdt:off + 3 * wdt],
                        xt[:, off:off + wdt], xt[:, off + wdt:off + 2 * wdt])
                    off += 2 * wdt
                xsum = spool.tile([128, V], f32, tag="xsum")
                nc.vector.tensor_add(xsum[:], xt[:, off:off + 25],
                                     xt[:, off + 25:off + 50])

                # x3 = W3blk @ X (v-major free order) in bf16
                x3sb = x3pool.tile([128, TV], bdt, tag="x3sb")
                for j in range(13):
                    w = 2 if j < 12 else 1
                    ps = pp.tile([128, 512], f32, tag="x3ps")
                    if cfg["load_split"]:
                        # per t-half matmuls so compute starts mid-load
                        psv = ps[:, 0:256 * w].rearrange(
                            "p (v h t) -> p v h t", v=w, h=2)
                        for h in range(2):
                            nc.tensor.matmul(
                                psv[:, :, h, :],
                                w3sb[:],
                                Xbv[:, 2 * j:2 * j + w, 128 * h:128 * h + 128],
                                start=True, stop=True)
                    else:
                        nc.tensor.matmul(ps[:, 0:256 * w],
                                         w3sb[:],
                                         Xbv[:, 2 * j:2 * j + w, :],
                                         start=True, stop=True)
                    dst = x3sb[:, 512 * j:512 * j + 256 * w]
                    rot = (cfg["x3cp0"] if (p == 0 and cfg["x3cp0"])
                           else cfg["x3cp"])
                    _copy(rot[j % len(rot)], dst, ps[:, 0:256 * w])

                # branch projections: 4 blocks (x1m,x2m,q,k) all on
                # partitions 0-15 ((s,r)), split along free (25 each).
                # One psum tile per pair holds both bps (cols 640:740)
                # and adj (cols 0:625) to stay within 2 banks.
                bps = pb.tile([128, 768], f32, tag="badj")
                for b in range(4):
                    nc.tensor.matmul(bps[0:16, 640 + 25 * b:640 + 25 * b + 25],
                                     wbsb[:, 16 * b:16 * b + 16], xsum[:],
                                     start=True, stop=True)
                return x3sb, bps

            def x3w(p, x3sb, s):
                k = 2 * p + s
                sb = s * 64
                xe = cfg["x3w_eng"]
                if xe == "alt":
                    xe = ("sync", "scalar")[s]
                for i in range(5):
                    ng = 13 if i < 4 else 12
                    j0 = sb + 13 * i
                    _eng(xe).dma_start(
                        dram_ap(k, 25 * i * IMGW,
                                [[T, ng], [IMGW, V], [1, T]]),
                        x3sb[j0:j0 + ng, :]
                        .rearrange("g (v t) -> g v t", v=V))

            def stage_a2(p, x3sb, bps, after_s0=None):
                if cfg["x3w_where"] == "early":
                    x3w(p, x3sb, 0)
                    x3w(p, x3sb, 1)
                bsb = spool.tile([16, 100], f32, tag="bsb")
                nc.vector.tensor_add(bsb[:], bps[0:16, 640:740], bbsb[:])

                # d = tanh(x1m[u] - x2m[v]);  att = q[u]*k[v]  (free=(v,u))
                x1 = bsb[:, 0:25].unsqueeze(1).broadcast_to([16, V, V])
                x2 = bsb[:, 25:50].unsqueeze(2).broadcast_to([16, V, V])
                qq = bsb[:, 50:75].unsqueeze(1).broadcast_to([16, V, V])
                kk = bsb[:, 75:100].unsqueeze(2).broadcast_to([16, V, V])
                dd = spool.tile([16, 625], f32, tag="dd")
                ddv = dd[:].rearrange("p (v u) -> p v u", v=V)
                nc.vector.tensor_tensor(ddv, x1, x2,
                                        op=mybir.AluOpType.subtract)
                dt_ = spool.tile([16, 625], f32, tag="dt")
                nc.scalar.activation(dt_[:], dd[:],
                                     mybir.ActivationFunctionType.Tanh)
                at = spool.tile([16, 625], f32, tag="at")
                atv = at[:].rearrange("p (v u) -> p v u", v=V)
                nc.vector.tensor_tensor(atv, qq, kk, op=mybir.AluOpType.mult)
                ea = spool.tile([16, 625], f32, tag="ea")
                nc.scalar.activation(ea[:], at[:],
                                     mybir.ActivationFunctionType.Exp)
                den = spool.tile([16, V], f32, tag="den")
                nc.vector.tensor_reduce(
                    den[:], ea[:].rearrange("p (v u) -> p u v", v=V),
                    axis=mybir.AxisListType.X, op=mybir.AluOpType.add)
                rden = spool.tile([16, V], f32, tag="rden")
                nc.vector.reciprocal(rden[:], den[:])
                sm = spool.tile([16, 625], f32, tag="sm")
                nc.vector.tensor_tensor(
                    sm[:].rearrange("p (v u) -> p v u", v=V),
                    ea[:].rearrange("p (v u) -> p v u", v=V),
                    rden[:].unsqueeze(1).broadcast_to([16, V, V]),
                    op=mybir.AluOpType.mult)
                res = spool.tile([16, 625], f32, tag="res")
                nc.scalar.activation(res[:], sm[:],
                                     mybir.ActivationFunctionType.Tanh)
                st = spool.tile([16, 625], bf16, tag="st")
                nc.vector.tensor_add(st[:], dt_[:], res[:])

                # adj = W4blk @ s + (A + 2*b4)   [128,(v,u)], bf16 matmul
                aps_ = bps
                nc.tensor.matmul(aps_[:, 0:512], w4sb[:], st[:, 0:512],
                                 start=True, stop=True)
                nc.tensor.matmul(aps_[:, 512:625], w4sb[:], st[:, 512:625],
                                 start=True, stop=True)
                adjsb = spool.tile([128, 625], bdt, tag="adjsb")
                nc.vector.tensor_add(adjsb[:], aps_[:, 0:625], arsb[:])

                # adj restripe writes + readback into the img tile.
                for s in range(2):
                    k = 2 * p + s
                    sb = s * 64
                    it = img[k % NIMG]
                    if cfg["x3w_where"] != "early":
                        x3w(p, x3sb, s)
                    # DMA APs are limited to 3 dims, so one DMA per i-block.
                    for i in range(5):
                        ng = 13 if i < 4 else 12
                        j0 = sb + 13 * i
                        # adj diag blocks: col 3328 + 125g + 25i + u
                        ae = cfg["adjw_eng"]
                        if ae == "alt":
                            ae = ("sync", "scalar")[s]
                        _eng(ae).dma_start(
                            dram_ap(k, AOFF + 25 * i * IMGW + 25 * i,
                                    [[125, ng], [IMGW, V], [1, V]]),
                            adjsb[j0:j0 + ng, :]
                            .rearrange("g (v u) -> g v u", v=V))
                    if cfg["rb_split"] == "2way":
                        _eng(cfg["rb_eng"]).dma_start(
                            it[:, 0:AOFF],
                            dram_ap(k, 0, [[IMGW, 125], [1, AOFF]]))
                        _eng(cfg["rb_eng"]).dma_start(
                            it[:, AOFF:IMGW],
                            dram_ap(k, AOFF, [[IMGW, 125], [1, IMGW - AOFF]]))
                    elif cfg["rb_split"]:
                        # x3 halves first: they only wait on x3w, so they
                        # stream while adjw waits on the branch chain.
                        _eng(cfg["rb_eng"]).dma_start(
                            it[:, 0:1664],
                            dram_ap(k, 0, [[IMGW, 125], [1, 1664]]))
                        _eng(cfg["rb_eng"]).dma_start(
                            it[:, 1664:AOFF],
                            dram_ap(k, 1664, [[IMGW, 125], [1, AOFF - 1664]]))
                        _eng(cfg["rb_eng"]).dma_start(
                            it[:, AOFF:IMGW],
                            dram_ap(k, AOFF, [[IMGW, 125], [1, IMGW - AOFF]]))
                    else:
                        _eng(cfg["rb_eng"]).dma_start(it[:], scr[k % NSCR])
                    if s == 0 and after_s0 is not None:
                        after_s0()

            def stage_b(p, s):
                # aggregation + output for sample 2p+s.  One matmul per
                # o-group g: lhsT = adjT block (stationary), rhs = x3 img
                # rows -> psum [(i,u), t] with full t=256 moving dim.
                # osb mirrors the img x3 layout [125, (g,t)]; one output
                # DMA per sample, host undoes the (g,i,u,t) permutation.
                k = 2 * p + s
                it = img[k % NIMG]
                osb = opool.tile([125, 3328], bf16, tag="osb")
                for q in range(7):  # psum tiles of 2 groups
                    glist = range(2 * q, min(2 * q + 2, NG))
                    ag = pa.tile([128, 512], f32, tag="aggps")
                    for gi, g in enumerate(glist):
                        kg = 125 if g < 12 else 100
                        nc.tensor.matmul(
                            ag[0:kg, 256 * gi:256 * gi + T],
                            it[0:kg, AOFF + 125 * g:AOFF + 125 * g + kg],
                            it[0:kg, T * g:T * g + T],
                            start=True, stop=True)
                    w = 256 * len(glist)
                    dst = osb[:, 512 * q:512 * q + w]
                    rot = cfg["aggcp"]
                    _copy(rot[q % len(rot)], dst, ag[0:125, 0:w])
                    if cfg["out_split"] and q == 2:
                        _eng(cfg["out_eng"]).dma_start(out_d[k][:, 0:1536],
                                                       osb[:, 0:1536])
                if cfg["out_split"]:
                    _eng(cfg["out_eng"]).dma_start(out_d[k][:, 1536:3328],
                                                   osb[:, 1536:3328])
                else:
                    _eng(cfg["out_eng"]).dma_start(out_d[k], osb[:])

            load_pair(0)
            # one-time zero init of the adjT region of every scratch tensor
            # (off-diagonal blocks stay zero forever); after load 0 so the
            # first X transfer isn't queued behind them.
            nc.vector.memset(zt[:], 0.0)
            for k in range(NSCR):
                _eng(cfg.get("zi_eng", "scalar")).dma_start(
                    bass.AP(scr[k].tensor, AOFF, [[IMGW, 125], [1, 1600]]),
                    zt[0:125, 0:1600])
            for p in range(4):
                x3sb, bps = stage_a1(p)
                if p >= 1:
                    stage_b(p - 1, 0)
                    if not cfg["b_split"]:
                        stage_b(p - 1, 1)
                cb = (lambda: stage_b(3, 0)) \
                    if (p == 3 and cfg["tail_interleave"]) else None
                stage_a2(p, x3sb, bps, after_s0=cb)
                if p >= 1 and cfg["b_split"]:
                    stage_b(p - 1, 1)
            if not cfg["tail_interleave"]:
                stage_b(3, 0)
            stage_b(3, 1)
    nc.compile()
    return nc


def _get_nc():
    if "nc" not in _cache:
        _cache["nc"] = _build_nc()
    return _cache["nc"]


def _host_weights(A, W1, b1, W2, b2, W3, b3, W4, b4, W5, b5, w6, b6, w7, b7):
    f = np.float32
    s5 = np.sqrt(np.float32(5.0))
    Wq = (s5 * w6 * W5).astype(f)
    Wk = (s5 * w7 * W5).astype(f)
    # per-row biases
    bq = (s5 * (w6 * b5 + b6)).astype(f)  # [R]
    bk = (s5 * (w7 * b5 + b7)).astype(f)

    w3blk = np.zeros((128, 128), f)
    perm = np.array([_o_of_j(j) for j in range(64)])  # j -> o
    for s in range(2):
        # columns m = s*64+j hold o=perm[j]:  lhsT[k=c, m] = W3[o, c]
        w3blk[s * 64:(s + 1) * 64, s * 64:(s + 1) * 64] = W3[perm].T

    wbblk = np.zeros((128, 64), f)
    Wset = [W1, W2, Wq, Wk]
    for blk in range(4):
        for s in range(2):
            # columns 16*blk + s*8 + r ; rows k = s*64 + c
            wbblk[s * 64:(s + 1) * 64,
                  16 * blk + s * 8: 16 * blk + s * 8 + 8] = \
                (Wset[blk] / T).T
    bbvec = np.zeros((16, 100), f)
    bset = [b1, b2, bq, bk]
    for blk in range(4):
        for s in range(2):
            bbvec[s * 8:(s + 1) * 8, 25 * blk:25 * blk + 25] = \
                bset[blk][:, None]

    w4blk = np.zeros((16, 128), f)
    for s in range(2):
        # k = s*8 + r ; m = s*64 + j with o=perm[j]
        w4blk[s * 8:(s + 1) * 8, s * 64:(s + 1) * 64] = W4[perm].T

    arep = np.zeros((128, 625), f)
    avu = (A.T).reshape(-1)  # index v*25+u -> A[u,v]
    for s in range(2):
        for j in range(64):
            arep[s * 64 + j, :] = avu + 2.0 * b4[perm[j]]
    return w3blk, wbblk, w4blk, arep, bbvec


def kernel(**inputs):
    import jax  # noqa: F401  (ensures axon/jax devices initialized)
    import ml_dtypes
    from concourse.bass_utils import run_bass_kernel_spmd

    x = np.asarray(inputs["x"], np.float32)
    args = {k: np.asarray(np.float32(inputs[k]))
            for k in ["A", "W1", "b1", "W2", "b2", "W3", "b3", "W4", "b4",
                      "W5", "b5", "w6", "b6", "w7", "b7"]}
    w3blk, wbblk, w4blk, arep, bbvec = _host_weights(**args)
    xnp = (ml_dtypes.float8_e4m3fn if CFG["x_dt"] == "fp8"
           else ml_dtypes.bfloat16)
    w3blk = w3blk.astype(xnp)
    w4blk = w4blk.astype(ml_dtypes.bfloat16)

    nc = _get_nc()
    in_maps = []
    xb = x.astype(xnp)
    for core in range(NCORES):
        in_maps.append({
            "x": np.ascontiguousarray(xb[core * S:(core + 1) * S]),
            "w3blk": w3blk, "wbblk": wbblk, "w4blk": w4blk,
            "arep": arep, "bbvec": bbvec,
        })
    res = run_bass_kernel_spmd(nc, in_maps, list(range(NCORES)))
    _cache["last_results"] = res
    outs = []
    for core in range(NCORES):
        o = np.asarray(res.results[core]["out"]).astype(np.float32)
        # out[s, 25i+u, 256g+t] = y[s, o=5g+i, t, u]; slot o=64 is garbage
        a = o.reshape(S, 5, V, NG, T).transpose(0, 3, 1, 4, 2)
        outs.append(a.reshape(S, 65, T, V)[:, :64])  # -> [S, O, T, U]
    full = np.concatenate(outs, axis=0)

    # b3 correction: out += b3[o] * sum_v adj[o,u,v] — b3 is zero in this
    # problem's setup; assert to be explicit rather than silently wrong.
    assert not np.any(args["b3"]), "kernel assumes b3 == 0"
    return full.astype(np.float32)
